# revision 1
# baseline (speedup 1.0000x reference)
"""Center-contrast triplet loss on 8 Trainium2 NeuronCores.

Feature-dim sharding: core m gets the m-th 256-wide feature slice of both
inputs (shipped pre-transposed as [256, 4096] so the contraction dim lands on
SBUF partitions). Each core computes partial sum-centers s1/s2 via DVE
strided reduces, the partial Gram s1.T @ s2 on TensorE (float32r), and folds
the per-row / per-column bias terms (a_i = 0.5*|s2_i|^2 - s1_i.s2_i,
b_j = 0.5*|s2_j|^2) into the same PSUM accumulation as rank-1 matmuls, so the
PSUM holds the partial pre-relu "vals" matrix v = g + a_i - b_j directly.

One ReduceScatter(add) over the [512, 512] v buffer hands core m the summed
rows [64m, 64m+64); the core reduces them to per-row maxima. The host glue
gathers the 8x64 row maxima and finishes with the trivial relu/cummax/sum
epilogue (v is 32x the true vals because centers are kept as sums-of-8, so
the final scalar is divided by 32).
"""

import numpy as np

import concourse.bacc as bacc
import concourse.mybir as mybir
import concourse.tile as tile
from concourse.bass_utils import run_bass_kernel_spmd
from concourse.vector_clock import ScopedClock


class LeanTileContext(tile.TileContext):
    """TileContext with a drain-only exit.

    The stock exit emits drain + all-engine EVSEM barrier + semaphore
    clears + second barrier (~15us on silicon). The runtime re-arms
    semaphores at NEFF load/execute, so for this single-shot kernel a
    drain (which already waits on every engine's clock) is sufficient;
    verified correct across repeated executions of the same NEFF.
    """

    def _drain_and_barrier(self, tick_clock, wait_clock):
        drain_inst = self.nc.sync.drain()
        wait_clock.add_sem_waits(
            drain_inst.ins, ScopedClock({None: tick_clock.global_clock})
        )
        popped = self.nc._tile_sem_poison_stack.pop()
        assert popped is self._sem_poison
        sems = list(self.sems.allocated().values())
        sem_nums = [s.num if hasattr(s, "num") else s for s in sems]
        self.nc._state.prepend_free_semaphores(sem_nums)
        for poison_set in self.nc._tile_sem_poison_stack:
            poison_set.update(sem_nums)

N_CORES = 8
B, D, C, K = 4096, 2048, 512, 8
DS = D // N_CORES          # 256 features per core
CB = C // N_CORES          # 64 classes per ReduceScatter block
F32 = mybir.dt.float32
F32R = mybir.dt.float32r
F16 = mybir.dt.float16


def build_nc():
    nc = bacc.Bacc(
        "TRN2", target_bir_lowering=False, debug=False, num_devices=N_CORES
    )
    x1t = nc.dram_tensor("x1t", [DS, B], F16, kind="ExternalInput")
    sel = nc.dram_tensor("sel", [2 * CB, CB], F16, kind="ExternalInput")
    x2t = nc.dram_tensor("x2t", [DS, B], F16, kind="ExternalInput")
    out = nc.dram_tensor("out", [CB, 1], F32, kind="ExternalOutput")
    v_bounce = nc.dram_tensor("v_bounce", [C, C], F16)
    a2a_out = nc.dram_tensor("a2a_out", [C, C], F16)

    with LeanTileContext(nc) as tc:
        with (
            tc.tile_pool(name="sbuf", bufs=1) as pool,
            tc.tile_pool(name="psum", bufs=1, space="PSUM") as psum,
        ):
            # memset can't write f32r; memset f32 scratch, copy-round to f32r
            const_f32 = pool.tile([128, C + 128], F32, name="const_f32")
            nc.vector.memset(const_f32[:], 1.0)
            nc.vector.memset(const_f32[0:1, C : C + 128], -1.0)
            ones_col = pool.tile([128, 1], F32R, name="ones_col")
            nc.vector.tensor_copy(ones_col[:], const_f32[:, 0:1])
            ones_row = pool.tile([1, C], F32R, name="ones_row")
            nc.vector.tensor_copy(ones_row[:], const_f32[0:1, 0:C])
            neg_row = pool.tile([1, 128], F32R, name="neg_row")
            nc.vector.tensor_copy(neg_row[:], const_f32[0:1, C : C + 128])
            nc.vector.memset(const_f32[:, C + 126 : C + 128], 0.5)
            half_col2 = pool.tile([128, 2], F32R, name="half_col2")
            nc.vector.tensor_copy(half_col2[:], const_f32[:, C + 126 : C + 128])

            # tiny first DMA warms the HWDGE queue before the big stream
            warm_sb = pool.tile([1, 64], F16, name="warm_sb")
            nc.sync.dma_start(warm_sb[:], x2t[0:1, 0:64])
            sel_sb = pool.tile([2 * CB, CB], F16, name="sel_sb")
            nc.sync.dma_start(sel_sb[:], sel[:])

            # all k-sums run as pairwise fp16 add-trees on DVE (16-bit
            # tensor_tensor hits the DVE 2x mode, ~3x faster than a grouped
            # reduce); final round writes f32r for the PE
            def ksum_tree(src_ap, width, out_f32r, tag):
                v0 = src_ap.rearrange("p (n two) -> p n two", two=2)
                r1 = pool.tile([128, width // 2], F16, name=f"kt1_{tag}")
                nc.vector.tensor_tensor(
                    r1[:], v0[:, :, 0], v0[:, :, 1], op=mybir.AluOpType.add
                )
                v1 = r1[:].rearrange("p (n two) -> p n two", two=2)
                r2 = pool.tile([128, width // 4], F16, name=f"kt2_{tag}")
                nc.vector.tensor_tensor(
                    r2[:], v1[:, :, 0], v1[:, :, 1], op=mybir.AluOpType.add
                )
                v2 = r2[:].rearrange("p (n two) -> p n two", two=2)
                nc.vector.tensor_tensor(
                    out_f32r[:], v2[:, :, 0], v2[:, :, 1], op=mybir.AluOpType.add
                )

            x2 = {}
            for ch in range(2):
                t = pool.tile([128, B], F16, name=f"x2_{ch}")
                nc.sync.dma_start(t[:], x2t[128 * ch : 128 * (ch + 1), :])
                x2[ch] = t
            # x2 chunk 0 (lands first) sums on GpSimd, chunk 1 on Vector —
            # the two trees run in parallel
            s2T = {}
            with nc.allow_low_precision(reason="fp16 tree-sum for centers"):
                for ch, eng in ((0, nc.gpsimd), (1, nc.vector)):
                    v0 = x2[ch][:].rearrange("p (n two) -> p n two", two=2)
                    r1 = pool.tile([128, B // 2], F16, name=f"x2t1_{ch}")
                    eng.tensor_tensor(
                        r1[:], v0[:, :, 0], v0[:, :, 1], op=mybir.AluOpType.add
                    )
                    v1 = r1[:].rearrange("p (n two) -> p n two", two=2)
                    r2 = pool.tile([128, B // 4], F16, name=f"x2t2_{ch}")
                    eng.tensor_tensor(
                        r2[:], v1[:, :, 0], v1[:, :, 1], op=mybir.AluOpType.add
                    )
                    v2 = r2[:].rearrange("p (n two) -> p n two", two=2)
                    s = pool.tile([128, C], F32R, name=f"s2_{ch}")
                    eng.tensor_tensor(
                        s[:], v2[:, :, 0], v2[:, :, 1], op=mybir.AluOpType.add
                    )
                    s2T[ch] = s

            # x1: two big fp16 chunk loads; tree rounds 1-2 are chunk-wide
            # (fewer, bigger DVE ops), round 3 is per 128-class piece
            # x1 streams as four half-chunk loads so the last row-block's
            # tree tail is half as deep in elements (r1 on 2048-in, not 4096)
            HW = B // 2
            x1, s1p = {}, {}
            with nc.allow_low_precision(reason="fp16 tree-sum for centers"):
                for ch in range(2):
                    for hf in range(2):
                        t = pool.tile([128, HW], F16, name=f"x1_{ch}_{hf}")
                        nc.sync.dma_start(
                            t[:],
                            x1t[128 * ch : 128 * (ch + 1), HW * hf : HW * (hf + 1)],
                        )
                        v0 = t[:].rearrange("p (n two) -> p n two", two=2)
                        r1 = pool.tile([128, HW // 2], F16, name=f"x1t1_{ch}_{hf}")
                        nc.vector.tensor_tensor(
                            r1[:], v0[:, :, 0], v0[:, :, 1], op=mybir.AluOpType.add
                        )
                        v1 = r1[:].rearrange("p (n two) -> p n two", two=2)
                        r2 = pool.tile([128, HW // 4], F16, name=f"x1t2_{ch}_{hf}")
                        nc.vector.tensor_tensor(
                            r2[:], v1[:, :, 0], v1[:, :, 1], op=mybir.AluOpType.add
                        )
                        for j in range(2):
                            q = 2 * hf + j
                            v2 = r2[:, 256 * j : 256 * (j + 1)].rearrange(
                                "p (n two) -> p n two", two=2
                            )
                            s = pool.tile([128, 128], F32R, name=f"s1_{q}_{ch}")
                            nc.vector.tensor_tensor(
                                s[:], v2[:, :, 0], v2[:, :, 1],
                                op=mybir.AluOpType.add,
                            )
                            s1p[q, ch] = s

            # s2 stats: ss_j = |s2_j|^2 -> b_row (early, x2-only)
            ss_ps = psum.tile([1, C], F32, name="ss_ps")
            sqt = {}
            for ch in range(2):
                sq = pool.tile([128, C], F32R, name=f"sq{ch}")
                nc.vector.tensor_mul(sq[:], s2T[ch][:], s2T[ch][:])
                sqt[ch] = sq
                nc.tensor.matmul(
                    ss_ps[:], lhsT=ones_col[:], rhs=sq[:],
                    start=(ch == 0), stop=(ch == 1),
                )
            b_row = pool.tile([1, C], F32R, name="b_row")
            nc.vector.tensor_scalar_mul(b_row[:], ss_ps[:], 0.5)

            # per-block a_col[i] = 0.5*ss_i - dd_i accumulated in PSUM via
            # ones-matmuls with +0.5 / -1 constant columns (lhsT.T @ col);
            # the a-bias is then fused into the fp16 cast as a per-partition
            # tensor_scalar add, shortening each row-block chain
            for q in range(4):
                cs = slice(128 * q, 128 * (q + 1))
                acol_ps = psum.tile([128, 2], F32, name=f"acol{q}")
                for ch in range(2):
                    # prod = -2 * s1 * s2, so 0.5*(ss + prod-sum) = a_col
                    prod = pool.tile([128, 128], F32R, name=f"prod{q}_{ch}")
                    nc.vector.scalar_tensor_tensor(
                        prod[:], s1p[q, ch][:], -2.0, s2T[ch][:, cs],
                        op0=mybir.AluOpType.mult, op1=mybir.AluOpType.mult,
                    )
                    nc.tensor.matmul(
                        acol_ps[:], lhsT=sqt[ch][:, cs], rhs=half_col2[:],
                        start=(ch == 0), stop=False,
                    )
                    nc.tensor.matmul(
                        acol_ps[:], lhsT=prod[:], rhs=half_col2[:],
                        start=False, stop=(ch == 1),
                    )
                ac = pool.tile([128, 1], F32, name=f"a_sb{q}")
                nc.vector.tensor_copy(ac[:], acol_ps[:, 0:1])

                g_ps = psum.tile([128, C], F32, name=f"g{q}", tag="gps", bufs=2)
                for ch in range(2):
                    nc.tensor.matmul(
                        g_ps[:], lhsT=s1p[q, ch][:], rhs=s2T[ch][:],
                        start=(ch == 0), stop=False,
                    )
                nc.tensor.matmul(
                    g_ps[:], lhsT=neg_row[:], rhs=b_row[:],
                    start=False, stop=True,
                )
                v_sb = pool.tile([128, C], F16, name=f"v_sb{q}")
                nc.vector.tensor_scalar(
                    v_sb[:], g_ps[:], ac[:], None, op0=mybir.AluOpType.add
                )
                nc.sync.dma_start(v_bounce[128 * q : 128 * (q + 1), :], v_sb[:])

            nc.gpsimd.collective_compute(
                "AllToAll",
                mybir.AluOpType.bypass,
                replica_groups=[list(range(N_CORES))],
                ins=[v_bounce[:].opt()],
                outs=[a2a_out[:].opt()],
            )

            # sum the 8 received row-blocks on the PE: load them as 4
            # partition-stacked [128, C] pairs (DMA issue spread over the
            # three DMA-capable sequencers) and accumulate all four into one
            # PSUM tile with matmuls against a stacked two-identity selector
            # sel[p, i] = (p % CB == i), so psum[i, j] = sum_s blk_s[i, j]
            dma_engines = [nc.sync, nc.gpsimd, nc.scalar, nc.sync]
            acc_ps = psum.tile([CB, C], F32, name="acc_ps")
            for p in range(4):
                t = pool.tile([2 * CB, C], F16, name=f"pair{p}")
                dma_engines[p].dma_start(
                    t[:], a2a_out[2 * CB * p : 2 * CB * (p + 1), :]
                )
                nc.tensor.matmul(
                    acc_ps[:],
                    lhsT=sel_sb[:],
                    rhs=t[:],
                    start=(p == 0),
                    stop=(p == 3),
                )
            rm = pool.tile([CB, 1], F32, name="rm")
            nc.vector.reduce_max(out=rm[:], in_=acc_ps[:], axis=mybir.AxisListType.X)
            nc.sync.dma_start(out[:], rm[:])

    nc.finalize()
    return nc


def prepare_in_maps(input1, input2):
    x1 = np.asarray(input1, dtype=np.float32)
    x2 = np.asarray(input2, dtype=np.float32)
    sel = np.zeros((2 * CB, CB), dtype=np.float16)
    sel[np.arange(2 * CB), np.arange(2 * CB) % CB] = 1.0
    in_maps = []
    for m in range(N_CORES):
        sl = slice(m * DS, (m + 1) * DS)
        in_maps.append(
            {
                "x1t": np.ascontiguousarray(x1[:, sl].T, dtype=np.float16),
                "x2t": np.ascontiguousarray(x2[:, sl].T, dtype=np.float16),
                "sel": sel,
            }
        )
    return in_maps


def postprocess(results):
    rm = np.concatenate(
        [np.asarray(results[m]["out"]).reshape(CB) for m in range(N_CORES)]
    )
    rm = np.maximum(rm, 0.0) / 32.0
    return np.float32(np.maximum.accumulate(rm).sum())


_NC_CACHE = None


def kernel(input1, input2, targets1, targets2):
    global _NC_CACHE
    if _NC_CACHE is None:
        _NC_CACHE = build_nc()
    in_maps = prepare_in_maps(input1, input2)
    res = run_bass_kernel_spmd(_NC_CACHE, in_maps, list(range(N_CORES)))
    return postprocess(res.results)



# revision 5
# speedup vs baseline: 2.4441x; 2.4441x over previous
"""Center-contrast triplet loss on 8 Trainium2 NeuronCores — collective-free.

Feature-dim sharding: core m gets the m-th 256-wide feature slice of both
inputs, shipped pre-transposed as [256, 4096] fp16 with the batch axis
reordered to (k, c) so the per-class K-sum is three contiguous halving adds
on the DVE (unit stride, 16-bit 2x mode). Each core computes partial
sum-centers s1/s2, the partial Gram s1.T @ s2 on TensorE (fp16, f32 PSUM),
and the partial bias rows ss_j = sum_p s2^2 and pp_i = sum_p s1*s2 via
ones-matmuls.

No on-device collective: the ncfw rendezvous costs ~75us under this runtime,
dwarfing the 0.5MB of data. Instead every core DMAs its partial Gram
[512, 512] (fp16) plus the two bias rows (f32) straight to its output, and
the host unshard step sums the 8 partials, folds the biases, and runs the
trivial relu/rowmax/cummax/sum epilogue (all values are 32x the true vals
because centers are kept as sums-of-8; the final scalar is divided by 32).
"""

import numpy as np

import concourse.bacc as bacc
import concourse.mybir as mybir
import concourse.tile as tile
from concourse.bass_utils import run_bass_kernel_spmd
from concourse.vector_clock import ScopedClock


class LeanTileContext(tile.TileContext):
    """TileContext with a drain-only exit.

    The stock exit emits drain + all-engine EVSEM barrier + semaphore
    clears + second barrier (~15us on silicon). The runtime re-arms
    semaphores at NEFF load/execute, so for this single-shot kernel a
    drain (which already waits on every engine's clock) is sufficient;
    verified correct across repeated executions of the same NEFF.
    """

    def _drain_and_barrier(self, tick_clock, wait_clock):
        drain_inst = self.nc.sync.drain()
        wait_clock.add_sem_waits(
            drain_inst.ins, ScopedClock({None: tick_clock.global_clock})
        )
        popped = self.nc._tile_sem_poison_stack.pop()
        assert popped is self._sem_poison
        sems = list(self.sems.allocated().values())
        sem_nums = [s.num if hasattr(s, "num") else s for s in sems]
        self.nc._state.prepend_free_semaphores(sem_nums)
        for poison_set in self.nc._tile_sem_poison_stack:
            poison_set.update(sem_nums)


N_CORES = 8
B, D, C, K = 4096, 2048, 512, 8
DS = D // N_CORES          # 256 features per core
F32 = mybir.dt.float32
F16 = mybir.dt.float16


def build_nc():
    nc = bacc.Bacc(
        "TRN2", target_bir_lowering=False, debug=False, num_devices=N_CORES
    )
    x1t = nc.dram_tensor("x1t", [DS, B], F16, kind="ExternalInput")
    x2t = nc.dram_tensor("x2t", [DS, B], F16, kind="ExternalInput")
    v = nc.dram_tensor("v", [C, C], F16, kind="ExternalOutput")
    ab = nc.dram_tensor("ab", [1, 2 * C], F32, kind="ExternalOutput")

    with LeanTileContext(nc) as tc:
        with (
            tc.tile_pool(name="sbuf", bufs=1) as pool,
            tc.tile_pool(name="psum", bufs=1, space="PSUM") as psum,
        ):
            const_f32 = pool.tile([128, 1], F32, name="const_f32")
            nc.vector.memset(const_f32[:], 1.0)
            ones_col = pool.tile([128, 1], F16, name="ones_col")
            nc.vector.tensor_copy(ones_col[:], const_f32[:])

            # tiny first DMA warms the HWDGE queue before the big stream
            warm_sb = pool.tile([1, 64], F16, name="warm_sb")
            nc.sync.dma_start(warm_sb[:], x2t[0:1, 0:64])

            # big loads: x2 chunks first (everything needs s2), x1 after;
            # two HWDGE queues so the two chunks of each input stream in
            # parallel
            x2c, x1c = {}, {}
            for ch, eng in ((0, nc.sync), (1, nc.scalar)):
                t = pool.tile([128, B], F16, name=f"x2_{ch}")
                eng.dma_start(t[:], x2t[128 * ch : 128 * (ch + 1), :])
                x2c[ch] = t
            for ch, eng in ((0, nc.sync), (1, nc.scalar)):
                t = pool.tile([128, B], F16, name=f"x1_{ch}")
                eng.dma_start(t[:], x1t[128 * ch : 128 * (ch + 1), :])
                x1c[ch] = t

            # contiguous K-sum: batch cols are (k, c) ordered, so summing
            # 8 instances per class is three halving adds at unit stride
            def ksum(src, tag, eng):
                r1 = pool.tile([128, B // 2], F16, name=f"r1_{tag}")
                eng.tensor_tensor(
                    r1[:], src[:, : B // 2], src[:, B // 2 :],
                    op=mybir.AluOpType.add,
                )
                r2 = pool.tile([128, B // 4], F16, name=f"r2_{tag}")
                eng.tensor_tensor(
                    r2[:], r1[:, : B // 4], r1[:, B // 4 :],
                    op=mybir.AluOpType.add,
                )
                s = pool.tile([128, C], F16, name=f"s_{tag}")
                eng.tensor_tensor(
                    s[:], r2[:, :C], r2[:, C:], op=mybir.AluOpType.add
                )
                return s

            with nc.allow_low_precision(reason="fp16 tree-sum for centers"):
                s2 = {
                    0: ksum(x2c[0], "x2_0", nc.vector),
                    1: ksum(x2c[1], "x2_1", nc.gpsimd),
                }
                s1 = {
                    0: ksum(x1c[0], "x1_0", nc.vector),
                    1: ksum(x1c[1], "x1_1", nc.gpsimd),
                }

                # bias ingredient rows: ss_j = sum_p s2^2, pp_j = sum_p s1*s2
                ss_ps = psum.tile([1, C], F32, name="ss_ps")
                pp_ps = psum.tile([1, C], F32, name="pp_ps")
                for ch in range(2):
                    sq = pool.tile([128, C], F16, name=f"sq{ch}")
                    nc.vector.tensor_mul(sq[:], s2[ch][:], s2[ch][:])
                    nc.tensor.matmul(
                        ss_ps[:], lhsT=ones_col[:], rhs=sq[:],
                        start=(ch == 0), stop=(ch == 1),
                    )
                    pr = pool.tile([128, C], F16, name=f"pr{ch}")
                    nc.vector.tensor_mul(pr[:], s1[ch][:], s2[ch][:])
                    nc.tensor.matmul(
                        pp_ps[:], lhsT=ones_col[:], rhs=pr[:],
                        start=(ch == 0), stop=(ch == 1),
                    )

            ab_sb = pool.tile([1, 2 * C], F32, name="ab_sb")
            nc.vector.tensor_copy(ab_sb[:, 0:C], ss_ps[:])
            nc.vector.tensor_copy(ab_sb[:, C : 2 * C], pp_ps[:])
            nc.gpsimd.dma_start(ab[:], ab_sb[:])

            # partial Gram: g[i, j] = sum_f s1[f, i] * s2[f, j], built as
            # 4 row-blocks of 128 classes, 2 accumulating fp16 matmuls each
            out_eng = [nc.sync, nc.scalar, nc.sync, nc.scalar]
            for q in range(4):
                cs = slice(128 * q, 128 * (q + 1))
                g_ps = psum.tile([128, C], F32, name=f"g{q}", tag="gps", bufs=2)
                for ch in range(2):
                    nc.tensor.matmul(
                        g_ps[:], lhsT=s1[ch][:, cs], rhs=s2[ch][:],
                        start=(ch == 0), stop=(ch == 1),
                    )
                v_sb = pool.tile([128, C], F16, name=f"v_sb{q}")
                nc.vector.tensor_copy(v_sb[:], g_ps[:])
                out_eng[q].dma_start(v[128 * q : 128 * (q + 1), :], v_sb[:])

    nc.finalize()
    return nc


def prepare_in_maps(input1, input2):
    x1 = np.asarray(input1, dtype=np.float32)
    x2 = np.asarray(input2, dtype=np.float32)
    # [D, B] with batch reordered from (c, k) to (k, c): one big strided
    # gather per input, then per-core slices are contiguous views
    x1t = np.ascontiguousarray(
        x1.T.reshape(D, C, K).transpose(0, 2, 1), dtype=np.float16
    ).reshape(D, B)
    x2t = np.ascontiguousarray(
        x2.T.reshape(D, C, K).transpose(0, 2, 1), dtype=np.float16
    ).reshape(D, B)
    in_maps = []
    for m in range(N_CORES):
        sl = slice(m * DS, (m + 1) * DS)
        in_maps.append({"x1t": x1t[sl], "x2t": x2t[sl]})
    return in_maps


def postprocess(results):
    g = np.zeros((C, C), dtype=np.float32)
    ss = np.zeros(C, dtype=np.float64)
    pp = np.zeros(C, dtype=np.float64)
    for m in range(N_CORES):
        g += np.asarray(results[m]["v"], dtype=np.float32)
        a = np.asarray(results[m]["ab"], dtype=np.float64).reshape(2 * C)
        ss += a[:C]
        pp += a[C:]
    a_col = 0.5 * ss - pp          # per-row bias
    b_row = 0.5 * ss               # per-col bias
    vfull = g + (a_col[:, None] - b_row[None, :]).astype(np.float32)
    rm = np.maximum(vfull.max(axis=1), 0.0) / 32.0
    return np.float32(np.maximum.accumulate(rm).sum())


_NC_CACHE = None


def kernel(input1, input2, targets1, targets2):
    global _NC_CACHE
    if _NC_CACHE is None:
        _NC_CACHE = build_nc()
    in_maps = prepare_in_maps(input1, input2)
    res = run_bass_kernel_spmd(_NC_CACHE, in_maps, list(range(N_CORES)))
    return postprocess(res.results)


# revision 6
# speedup vs baseline: 3.0372x; 1.2427x over previous
"""Center-contrast triplet loss on 8 Trainium2 NeuronCores — collective-free.

Feature-dim sharding: core m gets the m-th 256-wide feature slice of both
inputs, shipped pre-transposed as [256, 4096] fp16 with the batch axis
reordered to (k, c) so the per-class K-sum is three contiguous halving adds
on the DVE (unit stride). Each core computes partial sum-centers s1/s2, the
partial Gram s1.T @ s2 on TensorE (bf16 operands, f32 PSUM), and the partial
bias rows ss_j = sum_p s2^2 and pp_j = sum_p s1*s2 via ones-matmuls.

No on-device collective: the ncfw rendezvous costs ~75us under this runtime,
dwarfing the 0.5MB of data. Instead every core DMAs its partial Gram
[512, 512] (fp16) plus the two bias rows (f32) straight to its output, and
the host unshard step sums the 8 partials, folds the biases, and runs the
trivial relu/rowmax/cummax/sum epilogue (all values are 32x the true vals
because centers are kept as sums-of-8; the final scalar is divided by 32).

Engine split: Vector owns the K-sum trees and s1*s2 products (it is ~3x
faster than GpSimd on tensor_tensor); Scalar/ACT owns squares and the
PSUM->SBUF casts; loads interleave one chunk of each input per HWDGE queue
so compute streams behind the DMA.
"""

import numpy as np

import concourse.bacc as bacc
import concourse.mybir as mybir
import concourse.tile as tile
from concourse.bass_utils import run_bass_kernel_spmd
from concourse.vector_clock import ScopedClock


class LeanTileContext(tile.TileContext):
    """TileContext with a drain-only exit.

    The stock exit emits drain + all-engine EVSEM barrier + semaphore
    clears + second barrier. The runtime re-arms semaphores at NEFF
    load/execute, so for this single-shot kernel a drain (which already
    waits on every engine's clock) is sufficient; verified correct across
    repeated executions of the same NEFF.
    """

    def _drain_and_barrier(self, tick_clock, wait_clock):
        drain_inst = self.nc.sync.drain()
        wait_clock.add_sem_waits(
            drain_inst.ins, ScopedClock({None: tick_clock.global_clock})
        )
        popped = self.nc._tile_sem_poison_stack.pop()
        assert popped is self._sem_poison
        sems = list(self.sems.allocated().values())
        sem_nums = [s.num if hasattr(s, "num") else s for s in sems]
        self.nc._state.prepend_free_semaphores(sem_nums)
        for poison_set in self.nc._tile_sem_poison_stack:
            poison_set.update(sem_nums)


N_CORES = 8
B, D, C, K = 4096, 2048, 512, 8
DS = D // N_CORES          # 256 features per core
F32 = mybir.dt.float32
F16 = mybir.dt.float16
BF16 = mybir.dt.bfloat16


def build_nc():
    nc = bacc.Bacc(
        "TRN2", target_bir_lowering=False, debug=False, num_devices=N_CORES
    )
    x1t = nc.dram_tensor("x1t", [DS, B], F16, kind="ExternalInput")
    x2t = nc.dram_tensor("x2t", [DS, B], F16, kind="ExternalInput")
    v = nc.dram_tensor("v", [C, C], F16, kind="ExternalOutput")
    ab = nc.dram_tensor("ab", [1, 2 * C], F32, kind="ExternalOutput")

    with LeanTileContext(nc) as tc:
        with (
            tc.tile_pool(name="sbuf", bufs=1) as pool,
            tc.tile_pool(name="psum", bufs=1, space="PSUM") as psum,
        ):
            const_f32 = pool.tile([128, 1], F32, name="const_f32")
            nc.vector.memset(const_f32[:], 1.0)
            ones_col = pool.tile([128, 1], BF16, name="ones_col")
            nc.vector.tensor_copy(ones_col[:], const_f32[:])

            # tiny first DMA warms the HWDGE queue before the big stream
            warm_sb = pool.tile([1, 64], F16, name="warm_sb")
            nc.sync.dma_start(warm_sb[:], x2t[0:1, 0:64])

            # chunk loads: one chunk of EACH input per queue, so both an s2
            # and an s1 chunk land early and compute can start at ~50% of
            # the stream
            xs = {}   # (which_input, ch) -> tile
            for (inp, t_dram, ch, eng) in (
                (2, x2t, 0, nc.sync),
                (1, x1t, 0, nc.scalar),
                (2, x2t, 1, nc.sync),
                (1, x1t, 1, nc.scalar),
            ):
                t = pool.tile([128, B], F16, name=f"x{inp}_{ch}")
                eng.dma_start(t[:], t_dram[128 * ch : 128 * (ch + 1), :])
                xs[inp, ch] = t

            # contiguous K-sum: batch cols are (k, c) ordered, so summing
            # 8 instances per class is three halving adds at unit stride,
            # all on Vector (GpSimd is ~3x slower on tensor_tensor)
            def ksum(src, tag):
                r1 = pool.tile([128, B // 2], F16, name=f"r1_{tag}")
                nc.vector.tensor_tensor(
                    r1[:], src[:, : B // 2], src[:, B // 2 :],
                    op=mybir.AluOpType.add,
                )
                r2 = pool.tile([128, B // 4], F16, name=f"r2_{tag}")
                nc.vector.tensor_tensor(
                    r2[:], r1[:, : B // 4], r1[:, B // 4 :],
                    op=mybir.AluOpType.add,
                )
                s = pool.tile([128, C], BF16, name=f"s_{tag}")
                nc.vector.tensor_tensor(
                    s[:], r2[:, :C], r2[:, C:], op=mybir.AluOpType.add
                )
                return s

            s1, s2, sq, pr = {}, {}, {}, {}
            ss_ps = psum.tile([1, C], F32, name="ss_ps")
            pp_ps = psum.tile([1, C], F32, name="pp_ps")
            g_ps = [
                psum.tile([128, C], F32, name=f"g{q}", tag="gps", bufs=4)
                for q in range(4)
            ]
            with nc.allow_low_precision(reason="16-bit tree-sum for centers"):
                for ch in range(2):
                    s2[ch] = ksum(xs[2, ch], f"x2_{ch}")
                    s1[ch] = ksum(xs[1, ch], f"x1_{ch}")
                    # squares on ACT, products on Vector
                    sq[ch] = pool.tile([128, C], BF16, name=f"sq{ch}")
                    nc.scalar.square(sq[ch][:], s2[ch][:])
                    pr[ch] = pool.tile([128, C], BF16, name=f"pr{ch}")
                    nc.vector.tensor_mul(pr[ch][:], s1[ch][:], s2[ch][:])
                    # partial Gram row-blocks for this chunk
                    for q in range(4):
                        cs = slice(128 * q, 128 * (q + 1))
                        nc.tensor.matmul(
                            g_ps[q][:], lhsT=s1[ch][:, cs], rhs=s2[ch][:],
                            start=(ch == 0), stop=(ch == 1),
                        )
                    nc.tensor.matmul(
                        ss_ps[:], lhsT=ones_col[:], rhs=sq[ch][:],
                        start=(ch == 0), stop=(ch == 1),
                    )
                    nc.tensor.matmul(
                        pp_ps[:], lhsT=ones_col[:], rhs=pr[ch][:],
                        start=(ch == 0), stop=(ch == 1),
                    )

            # PSUM -> SBUF casts on ACT, DMAs spread over both HWDGE queues
            out_eng = [nc.sync, nc.scalar, nc.sync, nc.scalar]
            for q in range(4):
                v_sb = pool.tile([128, C], F16, name=f"v_sb{q}")
                nc.scalar.copy(v_sb[:], g_ps[q][:])
                out_eng[q].dma_start(v[128 * q : 128 * (q + 1), :], v_sb[:])

            ab_sb = pool.tile([1, 2 * C], F32, name="ab_sb")
            nc.vector.tensor_copy(ab_sb[:, 0:C], ss_ps[:])
            nc.vector.tensor_copy(ab_sb[:, C : 2 * C], pp_ps[:])
            nc.gpsimd.dma_start(ab[:], ab_sb[:])

    nc.finalize()
    return nc


def prepare_in_maps(input1, input2):
    x1 = np.asarray(input1, dtype=np.float32)
    x2 = np.asarray(input2, dtype=np.float32)
    # [D, B] with batch reordered from (c, k) to (k, c): one big strided
    # gather per input, then per-core slices are contiguous views
    x1t = np.ascontiguousarray(
        x1.T.reshape(D, C, K).transpose(0, 2, 1), dtype=np.float16
    ).reshape(D, B)
    x2t = np.ascontiguousarray(
        x2.T.reshape(D, C, K).transpose(0, 2, 1), dtype=np.float16
    ).reshape(D, B)
    in_maps = []
    for m in range(N_CORES):
        sl = slice(m * DS, (m + 1) * DS)
        in_maps.append({"x1t": x1t[sl], "x2t": x2t[sl]})
    return in_maps


def postprocess(results):
    g = np.zeros((C, C), dtype=np.float32)
    ss = np.zeros(C, dtype=np.float64)
    pp = np.zeros(C, dtype=np.float64)
    for m in range(N_CORES):
        g += np.asarray(results[m]["v"], dtype=np.float32)
        a = np.asarray(results[m]["ab"], dtype=np.float64).reshape(2 * C)
        ss += a[:C]
        pp += a[C:]
    a_col = 0.5 * ss - pp          # per-row bias
    b_row = 0.5 * ss               # per-col bias
    vfull = g + (a_col[:, None] - b_row[None, :]).astype(np.float32)
    rm = np.maximum(vfull.max(axis=1), 0.0) / 32.0
    return np.float32(np.maximum.accumulate(rm).sum())


_NC_CACHE = None


def kernel(input1, input2, targets1, targets2):
    global _NC_CACHE
    if _NC_CACHE is None:
        _NC_CACHE = build_nc()
    in_maps = prepare_in_maps(input1, input2)
    res = run_bass_kernel_spmd(_NC_CACHE, in_maps, list(range(N_CORES)))
    return postprocess(res.results)
